# revision 11
# baseline (speedup 1.0000x reference)
"""Trainium2 Bass kernel for nn_CADenseMul.

Math (see reference):
    chi  = sigmoid(context @ W + Bc)          # [B, R]
    s    = S * chi                            # [B, R]
    out  = ((inputs @ U) * s) @ V.T + bias    # [B, UNITS]

Strategy v4:
  - Data-parallel over batch B across 8 cores (B=4096 -> 512 rows/core).
  - All input loads on the sync ring as 7 prioritized transfers
    (wc -> xh0 -> ub -> xh1 -> vb -> xh2 -> xh3) so DMA queues stay
    saturated and arrival order matches consumption order.
  - PE warm-up matmuls run while the first transfer streams in, so the
    clock is ramped before real work; then one dense lag-one pipeline:
        S1, S2 j0, S2 j1, S3 b0, S2 j2, S3 b1, S2 j3, S3 b2, S3 b3
  - 2-bank PSUM tiles in stage 3 halve the PSUM->SBUF copies; 5 output
    stores on sync; Bc DMA + bias skipped when Bc is all zeros.
"""

import numpy as np
import ml_dtypes

import concourse.bass as bass
import concourse.tile as tile
from concourse import bacc, mybir
from concourse.bass_utils import run_bass_kernel_spmd

N_CORES = 8
B, D_IN, D_CTX, UNITS, R = 4096, 2048, 512, 2048, 256
BS = B // N_CORES        # 512 batch rows per core
KT_X = D_IN // 128       # 16
KT_C = D_CTX // 128      # 4
RT = R // 128            # 2
NBT = BS // 128          # 4 output batch tiles / stage-2 slices

ACT_DTYPE = "bf16"       # referenced by test.py
N_WARM = 4

_COMPILED = {}


def _build(with_bc):
    dt = mybir.dt.bfloat16
    f32 = mybir.dt.float32
    nc = bacc.Bacc("TRN2", target_bir_lowering=False, debug=False,
                   num_devices=N_CORES)

    wc = nc.dram_tensor("wc", [128, KT_C * R + KT_C * BS], dt,
                        kind="ExternalInput").ap()          # W | ctxT
    xh = [nc.dram_tensor(f"xh{j}", [128, KT_X * 128], dt,
                         kind="ExternalInput").ap() for j in range(NBT)]
    ub = nc.dram_tensor("ub", [128, KT_X * R], dt,
                        kind="ExternalInput").ap()          # U_s
    vb = nc.dram_tensor("vb", [128, RT * UNITS], dt,
                        kind="ExternalInput").ap()          # V.T repacked
    if with_bc:
        Bc2 = nc.dram_tensor("Bc2", [128, RT], f32,
                             kind="ExternalInput").ap()
    out = nc.dram_tensor("out", [BS, UNITS], dt, kind="ExternalOutput").ap()

    W_off = 0
    ctx_off = KT_C * R

    with tile.TileContext(nc) as tc:
        with (
            tc.tile_pool(name="consts", bufs=1) as consts,
            tc.tile_pool(name="osb", bufs=4) as osb,
            tc.tile_pool(name="ps_h", bufs=RT, space="PSUM") as ps_h,
            tc.tile_pool(name="ps_p", bufs=2, space="PSUM") as ps_p,
            tc.tile_pool(name="ps_o", bufs=4, space="PSUM") as ps_o,
        ):
            # ---- loads: all on sync ring, consumption order ----
            wc_sb = consts.tile([128, KT_C * R + KT_C * BS], dt, tag="wc")
            nc.sync.dma_start(wc_sb[:], wc[:])
            xh_sb = []
            for j in range(NBT):
                xt = consts.tile([128, KT_X * 128], dt, tag=f"xh{j}")
                xh_sb.append(xt)
            ub_sb = consts.tile([128, KT_X * R], dt, tag="ub")
            vb_sb = consts.tile([128, RT * UNITS], dt, tag="vb")
            nc.sync.dma_start(xh_sb[0][:], xh[0][:])
            nc.sync.dma_start(ub_sb[:], ub[:])
            nc.sync.dma_start(xh_sb[1][:], xh[1][:])
            nc.sync.dma_start(vb_sb[:], vb[:])
            nc.sync.dma_start(xh_sb[2][:], xh[2][:])
            nc.sync.dma_start(xh_sb[3][:], xh[3][:])
            if with_bc:
                Bc_sb = consts.tile([128, RT], f32, tag="bc")
                nc.scalar.dma_start(Bc_sb[:], Bc2[:])

            chi_sb = consts.tile([128, RT * BS], f32, tag="chi")
            psT_sb = consts.tile([128, RT * BS], dt, tag="psT")

            # ---- PE warm-up on garbage data while wc streams in ----
            warm_sb = consts.tile([128, 512], dt, tag="warm")
            nc.gpsimd.memset(warm_sb[:], 0.0)
            warm_ps = ps_h.tile([128, 512], f32, tag="hps")
            for _ in range(N_WARM):
                nc.tensor.matmul(warm_ps[:], warm_sb[:, :128],
                                 warm_sb[:], start=True, stop=True)

            # ---- stage 1: h.T = W.T @ ctx.T ; chi = sigmoid(h + Bc) ----
            for rh in range(RT):
                ps = ps_h.tile([128, BS], f32, tag="hps")
                for n in range(KT_C):
                    nc.tensor.matmul(
                        ps[:],
                        wc_sb[:, W_off + n * R + rh * 128:
                              W_off + n * R + rh * 128 + 128],
                        wc_sb[:, ctx_off + n * BS: ctx_off + (n + 1) * BS],
                        start=(n == 0), stop=(n == KT_C - 1))
                if with_bc:
                    nc.scalar.activation(
                        chi_sb[:, rh * BS:(rh + 1) * BS], ps[:],
                        mybir.ActivationFunctionType.Sigmoid,
                        bias=Bc_sb[:, rh:rh + 1])
                else:
                    nc.scalar.activation(
                        chi_sb[:, rh * BS:(rh + 1) * BS], ps[:],
                        mybir.ActivationFunctionType.Sigmoid)

            # ---- stage 2 slice j: proj.T ; psT = proj.T * chi.T (bf16) ----
            def emit_s2(j):
                for rh in range(RT):
                    ps = ps_p.tile([128, 128], f32, tag="pps")
                    for k in range(KT_X):
                        nc.tensor.matmul(
                            ps[:],
                            ub_sb[:, k * R + rh * 128: k * R + rh * 128 + 128],
                            xh_sb[j][:, k * 128: (k + 1) * 128],
                            start=(k == 0), stop=(k == KT_X - 1))
                    nc.vector.tensor_mul(
                        psT_sb[:, rh * BS + j * 128: rh * BS + j * 128 + 128],
                        ps[:],
                        chi_sb[:, rh * BS + j * 128: rh * BS + j * 128 + 128])

            # ---- stage 3 tile bt: 2-bank PSUM; copies split vec/act ----
            def emit_s3(bt):
                o_sb = osb.tile([128, UNITS], dt, tag="o_sb")
                last = bt == NBT - 1
                for q in range(4):
                    ps = ps_o.tile([128, 512], f32, tag="ops")
                    vcol = (q // 2) * 2048 + (q % 2) * 512
                    for rh in range(RT):
                        nc.tensor.matmul(
                            ps[:],
                            psT_sb[:, rh * BS + bt * 128:
                                   rh * BS + bt * 128 + 128],
                            vb_sb[:, vcol + rh * 1024:
                                  vcol + rh * 1024 + 512],
                            start=(rh == 0), stop=(rh == RT - 1))
                    dst = o_sb[:, q * 512:(q + 1) * 512]
                    if q % 2:
                        nc.scalar.activation(
                            dst, ps[:], mybir.ActivationFunctionType.Copy)
                    else:
                        nc.vector.tensor_copy(dst, ps[:])
                    if last:
                        nc.sync.dma_start(
                            out[bt * 128:(bt + 1) * 128,
                                q * 512:(q + 1) * 512], dst)
                    elif q == 3:
                        nc.sync.dma_start(out[bt * 128:(bt + 1) * 128, :],
                                          o_sb[:])

            # lag-one pipeline: S3 of slice j runs one slice behind S2
            emit_s2(0)
            emit_s2(1)
            emit_s3(0)
            emit_s2(2)
            emit_s3(1)
            emit_s2(3)
            emit_s3(2)
            emit_s3(3)

    nc.compile()
    return nc


def _get_nc(key):
    if key not in _COMPILED:
        _COMPILED[key] = _build(key)
    return _COMPILED[key]


def _pack(a, p=128):
    """[n*p, m] row-major -> [p, n*m]: partition p holds rows p, p+128, ..."""
    n = a.shape[0] // p
    return np.ascontiguousarray(
        a.reshape(n, p, a.shape[1]).transpose(1, 0, 2).reshape(p, -1))


def _prep_in_maps(inputs, context, U, S, V, W, Bc, with_bc):
    np_act = ml_dtypes.bfloat16

    Us = np.asarray(U, np.float32) * np.asarray(S, np.float32)[None, :]
    ub = _pack(Us).astype(np_act)
    # vb repacked units-half-major: col = uh*2048 + rh*1024 + uu
    vb = _pack(np.ascontiguousarray(np.asarray(V, np.float32).T))
    vb = np.ascontiguousarray(
        vb.reshape(128, RT, 2, UNITS // 2).transpose(0, 2, 1, 3)
          .reshape(128, RT * UNITS)).astype(np_act)
    W32 = np.asarray(W, np.float32)

    x = np.asarray(inputs, np.float32)
    ctx = np.asarray(context, np.float32)
    in_maps = []
    for c in range(N_CORES):
        ctxT = ctx[c * BS:(c + 1) * BS, :].T
        wcb = np.concatenate([_pack(W32), _pack(np.ascontiguousarray(ctxT))],
                             axis=1).astype(np_act)
        xT = x[c * BS:(c + 1) * BS, :].T
        m = {"wc": wcb, "ub": ub, "vb": vb}
        if with_bc:
            m["Bc2"] = np.ascontiguousarray(
                np.asarray(Bc, np.float32).reshape(RT, 128).T)
        for j in range(NBT):
            m[f"xh{j}"] = _pack(np.ascontiguousarray(
                xT[:, j * 128:(j + 1) * 128])).astype(np_act)
        in_maps.append(m)
    return in_maps


def kernel(inputs, context, U, S, V, W, Bc, bias, _run_kwargs=None):
    with_bc = bool(np.any(np.asarray(Bc, np.float32)))
    nc = _get_nc(with_bc)
    in_maps = _prep_in_maps(inputs, context, U, S, V, W, Bc, with_bc)
    res = run_bass_kernel_spmd(nc, in_maps, list(range(N_CORES)),
                               **(_run_kwargs or {}))
    if _run_kwargs:
        kernel.last_results = res
    out = np.concatenate([np.asarray(res.results[c]["out"]).astype(np.float32)
                          for c in range(N_CORES)], axis=0)
    out += np.asarray(bias, np.float32)[None, :]
    return out
